# revision 17
# baseline (speedup 1.0000x reference)
"""VQ-codebook encoding layer kernel for Trainium2 (8 NeuronCores).

Math (per batch row n):
    smooth[t,k] = scale[k] * (||x_t||^2 - 2<x_t, c_k> + ||c_k||^2)
    A = softmax_k(smooth)
    E[k,d] = sum_t A[t,k] * x[t,d]  -  (sum_t A[t,k]) * c[k,d]

Sharding: data-parallel over N across 8 cores (8 rows each), codebook +
scale replicated. No collectives needed (forward only).

v3 design notes (from NTFF trace of v2, 118.5us):
  - v2 was Vector-bound (DVE 84us busy, 75% occupancy; PE starved with
    45us of >500ns stalls waiting on softmax results).  v3 rebalances:
    square -> ACT, xT psum->sbuf copy -> one big int32-bitcast ACT copy
    per half-row, vv/an -> GpSimd, recip -> custom-DVE approx (5x).
  - x loads become per-partition-contiguous (1 descriptor/partition vs
    32) to lift the SWDGE cast-DMA read rate from 290 GB/s toward HBM
    line rate; the pad column is dropped, so sum_t A comes from a
    ones-weights matmul accumulated into a step-0 PSUM region and a
    [1,K]->[K,1] mini-transpose at row end.
  - beta_k = scale_k*||c_k||^2 <= 2e-4 dropped entirely (exp factor
    within bf16 noise), as in v2.
"""

import numpy as np

import concourse.bass as bass
import concourse.bacc as bacc
import concourse.tile as tile
from concourse import mybir
from concourse import bass_utils
from concourse.masks import make_identity

N, T, K, D = 64, 4096, 32, 128
NCORES = 8
NP = N // NCORES          # rows per core
P = 128                   # partitions / token tile size
NTILES = T // P           # 32 token tiles per row
HT = NTILES // 2          # 16 token tiles per half-row unit

FP32 = mybir.dt.float32
BF16 = mybir.dt.bfloat16
U32 = mybir.dt.uint32

# sum_t A accumulates into PSUM via a step-0 output AP (one matmul per
# unit); fallback emits one small matmul per tile instead.
SUMA_STEP0 = True

DBG = None  # debug-dump hooks (see debug_kernel.py)


def _build_bass():
    nc = bacc.Bacc("TRN2", target_bir_lowering=False, num_swdge_queues=4)
    x = nc.dram_tensor("x", (NP, T, D), FP32, kind="ExternalInput")
    cw = nc.dram_tensor("codewords", (K, D), FP32, kind="ExternalInput")
    sc = nc.dram_tensor("scale", (K,), FP32, kind="ExternalInput")
    out = nc.dram_tensor("out", (NP, K, D), FP32, kind="ExternalOutput")

    with tile.TileContext(nc) as tc:
        _kernel_body(tc, out[:], x[:], cw[:], sc[:])
    nc.compile()
    return nc


def _kernel_body(tc, out, x, cw, sc):
    nc = tc.nc
    MULT = mybir.AluOpType.mult
    ADD = mybir.AluOpType.add
    AXX = mybir.AxisListType.X
    EXP = mybir.ActivationFunctionType.Exp
    SQUARE = mybir.ActivationFunctionType.Square

    with (
        tc.tile_pool(name="consts", bufs=1) as consts,
        tc.tile_pool(name="xload", bufs=5) as xload,
        tc.tile_pool(name="xtp", bufs=3) as xtp,
        tc.tile_pool(name="sqp", bufs=2) as sqp,
        tc.tile_pool(name="soft", bufs=3) as soft,
        tc.tile_pool(name="outp", bufs=2) as outp,
        tc.tile_pool(name="ptr", bufs=2, space="PSUM") as ptr,
        tc.tile_pool(name="pq", bufs=2, space="PSUM") as pq,
        tc.tile_pool(name="pe", bufs=2, space="PSUM") as pe_pool,
    ):
        # ---------------- setup (once) ----------------
        c_sb = consts.tile([K, D], FP32)          # c[k,d]
        nc.sync.dma_start(c_sb[:], cw)
        cT_sb = consts.tile([D, K], FP32)         # c^T[d,k]
        nc.sync.dma_start(cT_sb[:], cw.rearrange("k d -> d k"))
        scale_bc = consts.tile([P, K], FP32)      # scale[k] on 128 partitions
        nc.sync.dma_start(scale_bc[:], sc[None, :].to_broadcast((P, K)))

        # W[d,k] = -2 * scale_k * c^T  (bf16)
        W = consts.tile([D, K], BF16)
        nc.vector.scalar_tensor_tensor(
            out=W[:], in0=cT_sb[:], scalar=-2.0, in1=scale_bc[0:D, :],
            op0=MULT, op1=MULT,
        )

        ident = consts.tile([P, P], BF16)         # PE-transpose identity
        make_identity(nc, ident[:])
        ones_col = consts.tile([P, 1], BF16)      # sum_t A weights
        nc.vector.memset(ones_col[:], 1.0)
        ones11 = consts.tile([1, 1], BF16)        # mini-transpose moving op
        nc.vector.memset(ones11[:], 1.0)
        c_neg = consts.tile([K, D], FP32)         # -c for the final fixup
        nc.scalar.mul(c_neg[:], c_sb[:], -1.0)

        # ---------------- per-unit state ----------------
        units = [(n, h) for n in range(NP) for h in range(2)]
        xbfs = {}     # row  -> xbf [P, NTILES, D] bf16
        ptrs = {}     # unit -> psum transpose tile [D, HT, P] bf16
        xTs = {}      # unit -> xT sbuf [D, HT, P] bf16
        sqxs = {}     # unit -> sqx [P, HT] fp32
        vvs = {}      # unit -> vv [P, HT, K] fp32
        qns = {}      # unit -> qn psum [P, HT, K] fp32
        u8s = {}      # unit -> u8 [P, HT, K] bf16
        rinvs = {}    # unit -> rinv [P, HT] fp32
        ans = {}      # unit -> an [P, HT, K] bf16
        pes = {}      # row  -> psum E tile [K, 192] fp32

        def load_row(n, nsplit=1):
            xbf = xload.tile([P, NTILES, D], BF16)
            step = NTILES // nsplit
            for g in range(nsplit):
                nc.gpsimd.dma_start(
                    out=xbf[:, g * step : (g + 1) * step, :],
                    in_=x[n].rearrange("(p i) d -> p i d", p=P)[
                        :, g * step : (g + 1) * step, :
                    ],
                )
            xbfs[n] = xbf

        def phase_T(u):
            # PE: transpose the unit's 16 token tiles into one psum tile
            n, h = u
            xbf = xbfs[n]
            pt = ptr.tile([D, HT, P], BF16)
            for jj in range(HT):
                nc.tensor.transpose(
                    pt[:, jj, :], xbf[:, h * HT + jj, :], ident[:]
                )
            ptrs[u] = pt

        CP_U32 = False

        def phase_CP(u):
            # ACT: one bitcast copy psum -> sbuf for the whole unit
            pt = ptrs.pop(u)
            xT = xtp.tile([D, HT, P], BF16)
            if CP_U32:
                nc.scalar.copy(xT[:].bitcast(U32), pt[:].bitcast(U32))
            else:
                nc.scalar.copy(xT[:], pt[:])
            xTs[u] = xT
            if DBG and u == (0, 0):
                nc.gpsimd.dma_start(out=DBG["xT"], in_=xT[:])

        xsqs = {}     # unit -> xsq [P, HT, D] bf16

        def phase_SQ_act(u):
            # ACT: xsq = x*x (bf16), one iteration ahead of the DVE tree
            n, h = u
            xbf = xbfs[n]
            xsq = sqp.tile([P, HT, D], BF16, tag="xsq")
            nc.scalar.activation(
                xsq[:], xbf[:, h * HT : (h + 1) * HT, :], SQUARE
            )
            xsqs[u] = xsq

        def phase_SQ_dve(u):
            # DVE: fold 128 -> sqx
            xsq = xsqs.pop(u)
            f1 = sqp.tile([P, HT, 64], BF16, tag="f1")
            nc.vector.tensor_add(f1[:], xsq[:, :, 0:64], xsq[:, :, 64:128])
            f2 = sqp.tile([P, HT, 32], BF16, tag="f2")
            nc.vector.tensor_add(f2[:], f1[:, :, 0:32], f1[:, :, 32:64])
            f3 = sqp.tile([P, HT, 16], BF16, tag="f3")
            nc.vector.tensor_add(f3[:], f2[:, :, 0:16], f2[:, :, 16:32])
            sqx = sqp.tile([P, HT], FP32, tag="sqx")
            nc.vector.reduce_sum(sqx[:], f3[:], AXX)
            sqxs[u] = sqx
            if DBG and u == (0, 0):
                nc.gpsimd.dma_start(out=DBG["sqx"], in_=sqx[:])

        def phase_VV(u):
            # GPSIMD: vv[t,h,k] = sqx[t,h] * scale[k]
            sqx = sqxs.pop(u)
            vv = soft.tile([P, HT, K], FP32, tag="vv")
            nc.gpsimd.tensor_mul(
                vv[:],
                sqx[:, :, None].to_broadcast((P, HT, K)),
                scale_bc[:, None, :].to_broadcast((P, HT, K)),
            )
            vvs[u] = vv

        def phase_Q(u):
            # PE: cross-term matmuls qn[t,k] = -2 scale_k <x_t, c_k>
            xT = xTs.pop(u)
            qn = pq.tile([P, HT, K], FP32)
            for jj in range(HT):
                nc.tensor.matmul(
                    qn[:, jj, :], lhsT=xT[:, jj, :], rhs=W[:],
                    start=True, stop=True, skip_group_check=True,
                )
            qns[u] = qn

        def phase_QA(u):
            # DVE: qn += vv  (in psum)
            qn = qns[u]
            vv = vvs.pop(u)
            nc.vector.tensor_add(qn[:], qn[:], vv[:])
            if DBG and u == (0, 0):
                scr = outp.tile([P, HT, K], FP32, tag="dbgqn")
                nc.vector.tensor_copy(scr[:], qn[:])
                nc.gpsimd.dma_start(out=DBG["qn"], in_=scr[:])

        def phase_EX(u):
            # ACT: u8 = exp(qn)
            qn = qns.pop(u)
            u8 = soft.tile([P, HT, K], BF16, tag="u8")
            nc.scalar.activation(u8[:], qn[:], EXP)
            u8s[u] = u8
            if DBG and u == (0, 0):
                nc.gpsimd.dma_start(out=DBG["u8"], in_=u8[:])

        def phase_RS(u):
            # DVE: s = sum_k u8 ; rinv = 1/s
            u8 = u8s[u]
            s = sqp.tile([P, HT], FP32, tag="s")
            nc.vector.reduce_sum(s[:], u8[:], AXX)
            rinv = sqp.tile([P, HT], FP32, tag="rinv")
            nc.vector.reciprocal_approx_fast(rinv[:], s[:])
            rinvs[u] = rinv

        def phase_AN(u):
            # GPSIMD: an = u8 * rinv
            u8 = u8s.pop(u)
            rinv = rinvs.pop(u)
            an = soft.tile([P, HT, K], BF16, tag="an")
            nc.gpsimd.tensor_mul(
                an[:], u8[:], rinv[:, :, None].to_broadcast((P, HT, K))
            )
            ans[u] = an
            if DBG and u == (0, 0):
                nc.gpsimd.dma_start(out=DBG["an"], in_=an[:])

        def phase_E(u):
            n, h = u
            xbf = xbfs[n]
            an = ans.pop(u)
            if h == 0:
                pes[n] = pe_pool.tile([K, 192], FP32, name="psum_E", tag="pE")
            pe = pes[n]
            for jj in range(HT):
                nc.tensor.matmul(
                    pe[:, 0:D], lhsT=an[:, jj, :], rhs=xbf[:, h * HT + jj, :],
                    start=(h == 0 and jj == 0), stop=(h == 1 and jj == HT - 1),
                    skip_group_check=True,
                )
            # sum_t A[t,k] -> pe[0, 128+k], accumulated over both halves.
            # start=False always: a start=True here would clear the whole
            # PSUM bank's has_written bits and wipe the E accumulation
            # (observed on HW); the first E matmul's start=True clears the
            # bank once per row, covering this region too.
            if SUMA_STEP0:
                sa_out = pe[0:1, 128:160][:, None, :].to_broadcast((1, HT, K))
                nc.tensor.matmul(
                    sa_out, lhsT=ones_col[:], rhs=an[:],
                    start=False, stop=(h == 1), skip_group_check=True,
                )
            else:
                for jj in range(HT):
                    nc.tensor.matmul(
                        pe[0:1, 128:160], lhsT=ones_col[:], rhs=an[:, jj, :],
                        start=False, stop=(h == 1 and jj == HT - 1),
                        skip_group_check=True,
                    )
            if h == 1:
                finish_row(n)

        def finish_row(n):
            pe = pes.pop(n)
            xbfs.pop(n)
            if DBG and n == 0:
                scr = outp.tile([K, D], FP32, tag="dbgE")
                nc.vector.tensor_copy(scr[:], pe[:, 0:D])
                nc.gpsimd.dma_start(out=DBG["Eraw"], in_=scr[:])
                scr2 = outp.tile([1, 64], FP32, tag="dbgSA")
                nc.vector.tensor_copy(scr2[:, 0:32], pe[0:1, 128:160])
                nc.gpsimd.dma_start(out=DBG["sumA"], in_=scr2[:, 0:32])
            # [1,K] row of sums -> sbuf -> [K,1] column via mini-matmul
            sa_sb = outp.tile([1, K], BF16, tag="sa")
            nc.vector.tensor_copy(sa_sb[:], pe[0:1, 128:160])
            nc.tensor.matmul(
                pe[:, 160:161], lhsT=sa_sb[:], rhs=ones11[:],
                start=True, stop=True, skip_group_check=True,
            )
            # E[k,d] = raw - sumA_k * c[k,d]
            e_sb = outp.tile([K, D], FP32, tag="e")
            nc.vector.scalar_tensor_tensor(
                out=e_sb[:], in0=c_neg[:], scalar=pe[:, 160:161],
                in1=pe[:, 0:D], op0=MULT, op1=ADD,
            )
            nc.sync.dma_start(out[n], e_sb[:])

        # ---------------- software-pipelined main loop ----------------
        NU = len(units)
        load_row(0, nsplit=2)
        load_row(1)
        phase_SQ_act(units[0])
        for i, u in enumerate(units):
            n, h = u
            if h == 0 and n + 2 < NP:
                load_row(n + 2)
            # PE queue
            phase_T(u)
            if i >= 1:
                phase_Q(units[i - 1])
            if i >= 3:
                phase_E(units[i - 3])
            # ACT queue: next unit's square first, then exp, then copy
            if i + 1 < NU:
                phase_SQ_act(units[i + 1])
            if i >= 2:
                phase_EX(units[i - 2])
            phase_CP(u)
            # DVE queue
            phase_SQ_dve(u)
            if i >= 1:
                phase_QA(units[i - 1])
            if i >= 2:
                phase_RS(units[i - 2])
            # GPSIMD queue
            phase_VV(u)
            if i >= 2:
                phase_AN(units[i - 2])
        # tail (drain lagged phases)
        phase_Q(units[NU - 1])
        phase_EX(units[NU - 2])
        phase_QA(units[NU - 1])
        phase_RS(units[NU - 2])
        phase_AN(units[NU - 2])
        phase_E(units[NU - 3])
        phase_EX(units[NU - 1])
        phase_RS(units[NU - 1])
        phase_AN(units[NU - 1])
        phase_E(units[NU - 2])
        phase_E(units[NU - 1])


_NC_CACHE = None


def _get_nc():
    global _NC_CACHE
    if _NC_CACHE is None:
        _NC_CACHE = _build_bass()
    return _NC_CACHE


def kernel(**inputs):
    x = np.ascontiguousarray(np.asarray(inputs["x"], dtype=np.float32))
    cw = np.ascontiguousarray(np.asarray(inputs["codewords"], dtype=np.float32))
    sc = np.ascontiguousarray(np.asarray(inputs["scale"], dtype=np.float32))

    nc = _get_nc()
    in_maps = [
        {"x": x[i * NP : (i + 1) * NP], "codewords": cw, "scale": sc}
        for i in range(NCORES)
    ]
    res = bass_utils.run_bass_kernel_spmd(nc, in_maps, core_ids=list(range(NCORES)))
    return np.concatenate([r["out"] for r in res.results], axis=0)


if __name__ == "__main__":
    rng = np.random.default_rng(0)
    ins = {
        "x": rng.standard_normal((N, T, D), dtype=np.float32),
        "codewords": rng.uniform(-0.01, 0.01, (K, D)).astype(np.float32),
        "scale": rng.uniform(-0.01, 0.01, (K,)).astype(np.float32),
    }
    out = kernel(**ins)
    print(out.shape, out.dtype)

    # numpy reference check
    xx = ins["x"]; c = ins["codewords"]; s = ins["scale"]
    sqx = (xx * xx).sum(-1, keepdims=True)
    cross = xx @ c.T
    sqc = (c * c).sum(-1)
    sm = s * (sqx - 2 * cross + sqc)
    sm -= sm.max(-1, keepdims=True)
    A = np.exp(sm); A /= A.sum(-1, keepdims=True)
    E = np.einsum("ntk,ntd->nkd", A, xx) - A.sum(1)[:, :, None] * c
    err = np.abs(out - E).max() / np.abs(E).max()
    print("rel err vs numpy:", err)


# revision 19
# speedup vs baseline: 1.1686x; 1.1686x over previous
"""VQ-codebook encoding layer kernel for Trainium2 (8 NeuronCores).

Math (per batch row n):
    smooth[t,k] = scale[k] * (||x_t||^2 - 2<x_t, c_k> + ||c_k||^2)
    A = softmax_k(smooth)
    E[k,d] = sum_t A[t,k] * x[t,d]  -  (sum_t A[t,k]) * c[k,d]

Sharding: data-parallel over N across 8 cores (8 rows each), codebook +
scale replicated. No collectives needed (forward only).

v5 design notes (v2 118us Vector-bound; v3 121us ACT-bound):
  - The ||x||^2 * scale_k term is folded into the cross-term PSUM via a
    second matmul per tile: qn[t,k] = xT^T @ W + xsqT^T @ Wsq, where
    W[d,k] = -2 scale_k c[k,d] and Wsq[d,k] = scale_k.  xsqT = xT*xT is
    one 2x-mode DVE multiply per half-row.  This deletes the square,
    the 4-op fold tree, the sqx*scale outer product and the qn+=vv pass
    (~2.7us/unit of vector-engine time) for ~0.5us/unit of PE time
    (128-col LDWEIGHTS pipeline at ~27ns/MM pitch, measured).
  - x loads are per-partition-contiguous casting SWDGE DMAs (~313 GB/s
    read-side, near the ~358 HBM/NC limit).
  - sum_t A rides a ones-weights matmul into a step-0 PSUM region; the
    first E-matmul's start=True is the only bank-clear (start clears
    has_written for the whole PSUM bank, measured on HW).
  - beta_k = scale_k*||c_k||^2 <= 2e-4 dropped (within bf16 noise).
"""

import numpy as np

import concourse.bass as bass
import concourse.bacc as bacc
import concourse.tile as tile
from concourse import mybir
from concourse import bass_utils
from concourse.masks import make_identity

N, T, K, D = 64, 4096, 32, 128
NCORES = 8
NP = N // NCORES          # rows per core
P = 128                   # partitions / token tile size
NTILES = T // P           # 32 token tiles per row
HT = NTILES // 2          # 16 token tiles per half-row unit

FP32 = mybir.dt.float32
BF16 = mybir.dt.bfloat16
U32 = mybir.dt.uint32

SUMA_STEP0 = True
CP_U32 = False

DBG = None  # debug-dump hooks (see debug_kernel.py)


def _build_bass():
    nc = bacc.Bacc("TRN2", target_bir_lowering=False, num_swdge_queues=4)
    x = nc.dram_tensor("x", (NP, T, D), FP32, kind="ExternalInput")
    cw = nc.dram_tensor("codewords", (K, D), FP32, kind="ExternalInput")
    sc = nc.dram_tensor("scale", (K,), FP32, kind="ExternalInput")
    out = nc.dram_tensor("out", (NP, K, D), FP32, kind="ExternalOutput")

    with tile.TileContext(nc) as tc:
        _kernel_body(tc, out[:], x[:], cw[:], sc[:])
    nc.compile()
    return nc


def _kernel_body(tc, out, x, cw, sc):
    nc = tc.nc
    MULT = mybir.AluOpType.mult
    ADD = mybir.AluOpType.add
    AXX = mybir.AxisListType.X
    EXP = mybir.ActivationFunctionType.Exp

    with (
        tc.tile_pool(name="consts", bufs=1) as consts,
        tc.tile_pool(name="xload", bufs=5) as xload,
        tc.tile_pool(name="xtp", bufs=3) as xtp,
        tc.tile_pool(name="xsqp", bufs=2) as xsqp,
        tc.tile_pool(name="sqp", bufs=2) as sqp,
        tc.tile_pool(name="soft", bufs=3) as soft,
        tc.tile_pool(name="outp", bufs=2) as outp,
        tc.tile_pool(name="ptr", bufs=2, space="PSUM") as ptr,
        tc.tile_pool(name="pq", bufs=2, space="PSUM") as pq,
        tc.tile_pool(name="pe", bufs=2, space="PSUM") as pe_pool,
    ):
        # ---------------- setup (once) ----------------
        c_sb = consts.tile([K, D], FP32)          # c[k,d]
        nc.sync.dma_start(c_sb[:], cw)
        cT_sb = consts.tile([D, K], FP32)         # c^T[d,k]
        nc.sync.dma_start(cT_sb[:], cw.rearrange("k d -> d k"))
        scale_bc = consts.tile([P, K], FP32)      # scale[k] on 128 partitions
        nc.sync.dma_start(scale_bc[:], sc[None, :].to_broadcast((P, K)))

        # W[d,k] = -2 * scale_k * c^T  (bf16)
        W = consts.tile([D, K], BF16)
        nc.vector.scalar_tensor_tensor(
            out=W[:], in0=cT_sb[:], scalar=-2.0, in1=scale_bc[0:D, :],
            op0=MULT, op1=MULT,
        )
        # Wsq[d,k] = scale_k  (bf16) -- MM2 turns sum_d xsqT[d,t]*Wsq[d,k]
        # into scale_k * ||x_t||^2 accumulated straight into the qn psum.
        Wsq = consts.tile([D, K], BF16)
        nc.scalar.copy(Wsq[:], scale_bc[0:D, :])

        ident = consts.tile([P, P], BF16)         # PE-transpose identity
        make_identity(nc, ident[:])
        ones_col = consts.tile([P, 1], BF16)      # sum_t A weights
        nc.vector.memset(ones_col[:], 1.0)
        ones11 = consts.tile([1, 1], BF16)        # mini-transpose moving op
        nc.vector.memset(ones11[:], 1.0)
        c_neg = consts.tile([K, D], FP32)         # -c for the final fixup
        nc.scalar.mul(c_neg[:], c_sb[:], -1.0)

        # ---------------- per-unit state ----------------
        units = [(n, h) for n in range(NP) for h in range(2)]
        xbfs = {}     # row  -> xbf [P, NTILES, D] bf16
        ptrs = {}     # unit -> psum transpose tile [D, HT, P] bf16
        xTs = {}      # unit -> xT sbuf [D, HT, P] bf16
        xsqs = {}     # unit -> xsqT sbuf [D, HT, P] bf16
        qns = {}      # unit -> qn psum [P, HT, K] fp32
        u8s = {}      # unit -> u8 [P, HT, K] bf16
        rinvs = {}    # unit -> rinv [P, HT] fp32
        ans = {}      # unit -> an [P, HT, K] bf16
        pes = {}      # row  -> psum E tile [K, 192] fp32

        def load_row(n, nsplit=1):
            xbf = xload.tile([P, NTILES, D], BF16)
            step = NTILES // nsplit
            for g in range(nsplit):
                nc.gpsimd.dma_start(
                    out=xbf[:, g * step : (g + 1) * step, :],
                    in_=x[n].rearrange("(p i) d -> p i d", p=P)[
                        :, g * step : (g + 1) * step, :
                    ],
                )
            xbfs[n] = xbf

        def phase_T(u):
            # PE: transpose the unit's 16 token tiles into one psum tile
            n, h = u
            xbf = xbfs[n]
            pt = ptr.tile([D, HT, P], BF16)
            for jj in range(HT):
                nc.tensor.transpose(
                    pt[:, jj, :], xbf[:, h * HT + jj, :], ident[:]
                )
            ptrs[u] = pt

        def phase_CP(u):
            # ACT: one bitcast copy psum -> sbuf for the whole unit
            pt = ptrs.pop(u)
            xT = xtp.tile([D, HT, P], BF16)
            if CP_U32:
                nc.scalar.copy(xT[:].bitcast(U32), pt[:].bitcast(U32))
            else:
                nc.scalar.copy(xT[:], pt[:])
            xTs[u] = xT
            if DBG and u == (0, 0):
                nc.gpsimd.dma_start(out=DBG["xT"], in_=xT[:])

        def phase_XSQ(u):
            # DVE: xsqT = xT * xT (2x-mode bf16 multiply)
            xT = xTs[u]
            xsq = xsqp.tile([D, HT, P], BF16)
            nc.vector.tensor_mul(xsq[:], xT[:], xT[:])
            xsqs[u] = xsq

        def phase_Q(u):
            # PE: qn[t,k] = -2 scale_k <x_t, c_k> + scale_k ||x_t||^2
            xT = xTs.pop(u)
            xsq = xsqs.pop(u)
            qn = pq.tile([P, HT, K], FP32)
            for jj in range(HT):
                nc.tensor.matmul(
                    qn[:, jj, :], lhsT=xT[:, jj, :], rhs=W[:],
                    start=(jj == 0), stop=False, skip_group_check=True,
                )
            for jj in range(HT):
                nc.tensor.matmul(
                    qn[:, jj, :], lhsT=xsq[:, jj, :], rhs=Wsq[:],
                    start=False, stop=(jj == HT - 1), skip_group_check=True,
                )
            qns[u] = qn

        def phase_EX(u):
            # ACT: u8 = exp(qn)
            qn = qns.pop(u)
            u8 = soft.tile([P, HT, K], BF16, tag="u8")
            nc.scalar.activation(u8[:], qn[:], EXP)
            u8s[u] = u8
            if DBG and u == (0, 0):
                nc.gpsimd.dma_start(out=DBG["u8"], in_=u8[:])

        def phase_RS(u):
            # DVE: s = sum_k u8 ; rinv = 1/s
            u8 = u8s[u]
            s = sqp.tile([P, HT], FP32, tag="s")
            nc.vector.reduce_sum(s[:], u8[:], AXX)
            rinv = sqp.tile([P, HT], FP32, tag="rinv")
            nc.vector.reciprocal_approx_fast(rinv[:], s[:])
            rinvs[u] = rinv

        def phase_AN(u):
            # GPSIMD: an = u8 * rinv
            u8 = u8s.pop(u)
            rinv = rinvs.pop(u)
            an = soft.tile([P, HT, K], BF16, tag="an")
            nc.gpsimd.tensor_mul(
                an[:], u8[:], rinv[:, :, None].to_broadcast((P, HT, K))
            )
            ans[u] = an
            if DBG and u == (0, 0):
                nc.gpsimd.dma_start(out=DBG["an"], in_=an[:])

        def phase_E(u):
            n, h = u
            xbf = xbfs[n]
            an = ans.pop(u)
            if h == 0:
                pes[n] = pe_pool.tile([K, 192], FP32, name="psum_E", tag="pE")
            pe = pes[n]
            for jj in range(HT):
                nc.tensor.matmul(
                    pe[:, 0:D], lhsT=an[:, jj, :], rhs=xbf[:, h * HT + jj, :],
                    start=(h == 0 and jj == 0), stop=(h == 1 and jj == HT - 1),
                    skip_group_check=True,
                )
            # sum_t A[t,k] -> pe[0, 128+k], accumulated over both halves.
            # start=False always: a start=True here would clear the whole
            # PSUM bank's has_written bits and wipe the E accumulation
            # (observed on HW); the first E matmul's start=True clears the
            # bank once per row, covering this region too.
            if SUMA_STEP0:
                sa_out = pe[0:1, 128:160][:, None, :].to_broadcast((1, HT, K))
                nc.tensor.matmul(
                    sa_out, lhsT=ones_col[:], rhs=an[:],
                    start=False, stop=(h == 1), skip_group_check=True,
                )
            else:
                for jj in range(HT):
                    nc.tensor.matmul(
                        pe[0:1, 128:160], lhsT=ones_col[:], rhs=an[:, jj, :],
                        start=False, stop=(h == 1 and jj == HT - 1),
                        skip_group_check=True,
                    )
            if h == 1:
                finish_row(n)

        def finish_row(n):
            pe = pes.pop(n)
            xbfs.pop(n)
            if DBG and n == 0:
                scr = outp.tile([K, D], FP32, tag="dbgE")
                nc.vector.tensor_copy(scr[:], pe[:, 0:D])
                nc.gpsimd.dma_start(out=DBG["Eraw"], in_=scr[:])
                scr2 = outp.tile([1, 64], FP32, tag="dbgSA")
                nc.vector.tensor_copy(scr2[:, 0:32], pe[0:1, 128:160])
                nc.gpsimd.dma_start(out=DBG["sumA"], in_=scr2[:, 0:32])
            # [1,K] row of sums -> sbuf -> [K,1] column via mini-matmul
            sa_sb = outp.tile([1, K], BF16, tag="sa")
            nc.vector.tensor_copy(sa_sb[:], pe[0:1, 128:160])
            nc.tensor.matmul(
                pe[:, 160:161], lhsT=sa_sb[:], rhs=ones11[:],
                start=True, stop=True, skip_group_check=True,
            )
            # E[k,d] = raw - sumA_k * c[k,d]
            e_sb = outp.tile([K, D], FP32, tag="e")
            nc.vector.scalar_tensor_tensor(
                out=e_sb[:], in0=c_neg[:], scalar=pe[:, 160:161],
                in1=pe[:, 0:D], op0=MULT, op1=ADD,
            )
            nc.sync.dma_start(out[n], e_sb[:])

        # ---------------- software-pipelined main loop ----------------
        # per iteration i (unit u_i):
        #   PE : T(i)            Q+MM2(i-1)     E(i-3)
        #   ACT: EX(i-2)         CP(i)
        #   DVE: RS(i-2)         XSQ(i)   [finish via E(i-3)]
        #   GPS: [loads]         AN(i-2)
        NU = len(units)
        load_row(0, nsplit=2)
        load_row(1)
        for i, u in enumerate(units):
            n, h = u
            if h == 0 and n + 2 < NP:
                load_row(n + 2)
            # PE queue
            phase_T(u)
            if i >= 1:
                phase_Q(units[i - 1])
            if i >= 3:
                phase_E(units[i - 3])
            # ACT queue
            if i >= 2:
                phase_EX(units[i - 2])
            phase_CP(u)
            # DVE queue
            if i >= 2:
                phase_RS(units[i - 2])
            phase_XSQ(u)
            # GPSIMD queue
            if i >= 2:
                phase_AN(units[i - 2])
        # tail (drain lagged phases)
        phase_Q(units[NU - 1])
        phase_EX(units[NU - 2])
        phase_RS(units[NU - 2])
        phase_AN(units[NU - 2])
        phase_E(units[NU - 3])
        phase_EX(units[NU - 1])
        phase_RS(units[NU - 1])
        phase_AN(units[NU - 1])
        phase_E(units[NU - 2])
        phase_E(units[NU - 1])


_NC_CACHE = None


def _get_nc():
    global _NC_CACHE
    if _NC_CACHE is None:
        _NC_CACHE = _build_bass()
    return _NC_CACHE


def kernel(**inputs):
    x = np.ascontiguousarray(np.asarray(inputs["x"], dtype=np.float32))
    cw = np.ascontiguousarray(np.asarray(inputs["codewords"], dtype=np.float32))
    sc = np.ascontiguousarray(np.asarray(inputs["scale"], dtype=np.float32))

    nc = _get_nc()
    in_maps = [
        {"x": x[i * NP : (i + 1) * NP], "codewords": cw, "scale": sc}
        for i in range(NCORES)
    ]
    res = bass_utils.run_bass_kernel_spmd(nc, in_maps, core_ids=list(range(NCORES)))
    return np.concatenate([r["out"] for r in res.results], axis=0)


if __name__ == "__main__":
    rng = np.random.default_rng(0)
    ins = {
        "x": rng.standard_normal((N, T, D), dtype=np.float32),
        "codewords": rng.uniform(-0.01, 0.01, (K, D)).astype(np.float32),
        "scale": rng.uniform(-0.01, 0.01, (K,)).astype(np.float32),
    }
    out = kernel(**ins)
    print(out.shape, out.dtype)

    # numpy reference check
    xx = ins["x"]; c = ins["codewords"]; s = ins["scale"]
    sqx = (xx * xx).sum(-1, keepdims=True)
    cross = xx @ c.T
    sqc = (c * c).sum(-1)
    sm = s * (sqx - 2 * cross + sqc)
    sm -= sm.max(-1, keepdims=True)
    A = np.exp(sm); A /= A.sum(-1, keepdims=True)
    E = np.einsum("ntk,ntd->nkd", A, xx) - A.sum(1)[:, :, None] * c
    err = np.abs(out - E).max() / np.abs(E).max()
    print("rel err vs numpy:", err)


# revision 26
# speedup vs baseline: 1.4105x; 1.2070x over previous
"""VQ-codebook encoding layer kernel for Trainium2 (8 NeuronCores).

Math (per batch row n):
    smooth[t,k] = scale[k] * (||x_t||^2 - 2<x_t, c_k> + ||c_k||^2)
    A = softmax_k(smooth)
    E[k,d] = sum_t A[t,k] * x[t,d]  -  (sum_t A[t,k]) * c[k,d]

Sharding: data-parallel over N across 8 cores (8 rows each), codebook +
scale replicated. No collectives needed (forward only).

v5 design notes (v2 118us Vector-bound; v3 121us ACT-bound):
  - The ||x||^2 * scale_k term is folded into the cross-term PSUM via a
    second matmul per tile: qn[t,k] = xT^T @ W + xsqT^T @ Wsq, where
    W[d,k] = -2 scale_k c[k,d] and Wsq[d,k] = scale_k.  xsqT = xT*xT is
    one 2x-mode DVE multiply per half-row.  This deletes the square,
    the 4-op fold tree, the sqx*scale outer product and the qn+=vv pass
    (~2.7us/unit of vector-engine time) for ~0.5us/unit of PE time
    (128-col LDWEIGHTS pipeline at ~27ns/MM pitch, measured).
  - x loads are per-partition-contiguous casting SWDGE DMAs (~313 GB/s
    read-side, near the ~358 HBM/NC limit).
  - sum_t A rides a ones-weights matmul into a step-0 PSUM region; the
    first E-matmul's start=True is the only bank-clear (start clears
    has_written for the whole PSUM bank, measured on HW).
  - beta_k = scale_k*||c_k||^2 <= 2e-4 dropped (within bf16 noise).
"""

import numpy as np

import concourse.bass as bass
import concourse.bacc as bacc
import concourse.tile as tile
from concourse import mybir
from concourse import bass_utils
from concourse.masks import make_identity

N, T, K, D = 64, 4096, 32, 128
NCORES = 8
NP = N // NCORES          # rows per core
P = 128                   # partitions / token tile size
NTILES = T // P           # 32 token tiles per row
HT = NTILES // 2          # 16 token tiles per half-row unit

FP32 = mybir.dt.float32
BF16 = mybir.dt.bfloat16
U32 = mybir.dt.uint32

SUMA_STEP0 = True
CP_U32 = False

DBG = None  # debug-dump hooks (see debug_kernel.py)


def _build_bass():
    nc = bacc.Bacc("TRN2", target_bir_lowering=False, num_swdge_queues=4)
    x = nc.dram_tensor("x", (NP, T, D), FP32, kind="ExternalInput")
    cw = nc.dram_tensor("codewords", (K, D), FP32, kind="ExternalInput")
    cwT = nc.dram_tensor("codewordsT", (D, K), FP32, kind="ExternalInput")
    sc = nc.dram_tensor("scale", (K,), FP32, kind="ExternalInput")
    out = nc.dram_tensor("out", (NP, K, D), FP32, kind="ExternalOutput")

    with tile.TileContext(nc) as tc:
        _kernel_body(tc, out[:], x[:], cw[:], cwT[:], sc[:])
    nc.compile()
    return nc


def _kernel_body(tc, out, x, cw, cwT, sc):
    nc = tc.nc
    MULT = mybir.AluOpType.mult
    ADD = mybir.AluOpType.add
    AXX = mybir.AxisListType.X
    EXP = mybir.ActivationFunctionType.Exp

    with (
        tc.tile_pool(name="consts", bufs=1) as consts,
        tc.tile_pool(name="xload", bufs=6) as xload,
        tc.tile_pool(name="xtp", bufs=3) as xtp,
        tc.tile_pool(name="xsqp", bufs=2) as xsqp,
        tc.tile_pool(name="sqp", bufs=2) as sqp,
        tc.tile_pool(name="soft", bufs=3) as soft,
        tc.tile_pool(name="outp", bufs=2) as outp,
        tc.tile_pool(name="ptr", bufs=2, space="PSUM") as ptr,
        tc.tile_pool(name="pq", bufs=2, space="PSUM") as pq,
        tc.tile_pool(name="pe", bufs=2, space="PSUM") as pe_pool,
    ):
        # ---------------- setup (once) ----------------
        c_sb = consts.tile([K, D], FP32)          # c[k,d]
        nc.sync.dma_start(c_sb[:], cw)
        cT_sb = consts.tile([D, K], FP32)         # c^T[d,k] (host-transposed)
        nc.sync.dma_start(cT_sb[:], cwT)
        scale_bc = consts.tile([P, K], FP32)      # scale[k] on 128 partitions
        nc.sync.dma_start(scale_bc[:], sc[None, :].to_broadcast((P, K)))

        # W[d,k] = -2 * scale_k * c^T  (bf16)
        W = consts.tile([D, K], BF16)
        nc.vector.scalar_tensor_tensor(
            out=W[:], in0=cT_sb[:], scalar=-2.0, in1=scale_bc[0:D, :],
            op0=MULT, op1=MULT,
        )
        # Wsq[d,k] = scale_k  (bf16) -- MM2 turns sum_d xsqT[d,t]*Wsq[d,k]
        # into scale_k * ||x_t||^2 accumulated straight into the qn psum.
        Wsq = consts.tile([D, K], BF16)
        nc.scalar.copy(Wsq[:], scale_bc[0:D, :])

        ident = consts.tile([P, P], BF16)         # PE-transpose identity
        make_identity(nc, ident[:])
        ones_col = consts.tile([P, 1], BF16)      # sum_t A weights
        nc.vector.memset(ones_col[:], 1.0)
        ones11 = consts.tile([1, 1], BF16)        # mini-transpose moving op
        nc.vector.memset(ones11[:], 1.0)
        c_neg = consts.tile([K, D], FP32)         # -c for the final fixup
        nc.scalar.mul(c_neg[:], c_sb[:], -1.0)

        # ---------------- per-unit state ----------------
        units = [(n, h) for n in range(NP) for h in range(2)]
        xbfs = {}     # row  -> xbf [P, NTILES, D] bf16
        ptrs = {}     # unit -> psum transpose tile [D, HT, P] bf16
        xTs = {}      # unit -> xT sbuf [D, HT, P] bf16
        xsqs = {}     # unit -> xsqT sbuf [D, HT, P] bf16
        qns = {}      # unit -> qn psum [P, HT, K] fp32
        u8s = {}      # unit -> u8 [P, HT, K] bf16
        rinvs = {}    # unit -> rinv [P, HT] fp32
        ans = {}      # unit -> an [P, HT, K] bf16
        pes = {}      # row  -> psum E tile [K, 192] fp32

        def load_row(n, nsplit=1):
            xbf = xload.tile([P, NTILES, D], BF16)
            step = NTILES // nsplit
            for g in range(nsplit):
                nc.gpsimd.dma_start(
                    out=xbf[:, g * step : (g + 1) * step, :],
                    in_=x[n].rearrange("(p i) d -> p i d", p=P)[
                        :, g * step : (g + 1) * step, :
                    ],
                )
            xbfs[n] = xbf

        def phase_T(u):
            # PE: transpose the unit's 16 token tiles into one psum tile
            n, h = u
            xbf = xbfs[n]
            pt = ptr.tile([D, HT, P], BF16)
            for jj in range(HT):
                nc.tensor.transpose(
                    pt[:, jj, :], xbf[:, h * HT + jj, :], ident[:]
                )
            ptrs[u] = pt

        def phase_CP(u):
            # ACT: one bitcast copy psum -> sbuf for the whole unit
            pt = ptrs.pop(u)
            xT = xtp.tile([D, HT, P], BF16)
            if CP_U32:
                nc.scalar.copy(xT[:].bitcast(U32), pt[:].bitcast(U32))
            else:
                nc.scalar.copy(xT[:], pt[:])
            xTs[u] = xT
            if DBG and u == (0, 0):
                nc.gpsimd.dma_start(out=DBG["xT"], in_=xT[:])

        def phase_XSQ(u):
            # DVE: xsqT = xT * xT (2x-mode bf16 multiply)
            xT = xTs[u]
            xsq = xsqp.tile([D, HT, P], BF16)
            nc.vector.tensor_mul(xsq[:], xT[:], xT[:])
            xsqs[u] = xsq

        def phase_Q(u):
            # PE: qn[t,k] = -2 scale_k <x_t, c_k> + scale_k ||x_t||^2
            xT = xTs.pop(u)
            xsq = xsqs.pop(u)
            qn = pq.tile([P, HT, K], FP32)
            for jj in range(HT):
                nc.tensor.matmul(
                    qn[:, jj, :], lhsT=xT[:, jj, :], rhs=W[:],
                    start=(jj == 0), stop=False, skip_group_check=True,
                )
            for jj in range(HT):
                nc.tensor.matmul(
                    qn[:, jj, :], lhsT=xsq[:, jj, :], rhs=Wsq[:],
                    start=False, stop=(jj == HT - 1), skip_group_check=True,
                )
            qns[u] = qn

        def phase_EX(u):
            # ACT: u8 = exp(qn)
            qn = qns.pop(u)
            u8 = soft.tile([P, HT, K], BF16, tag="u8")
            nc.scalar.activation(u8[:], qn[:], EXP)
            u8s[u] = u8
            if DBG and u == (0, 0):
                nc.gpsimd.dma_start(out=DBG["u8"], in_=u8[:])

        def phase_RS(u):
            # DVE: s = sum_k u8 ; rinv = 1/s
            u8 = u8s[u]
            s = sqp.tile([P, HT], FP32, tag="s")
            nc.vector.reduce_sum(s[:], u8[:], AXX)
            rinv = sqp.tile([P, HT], FP32, tag="rinv")
            nc.vector.reciprocal_approx_fast(rinv[:], s[:])
            rinvs[u] = rinv

        def phase_AN(u):
            # GPSIMD: an = u8 * rinv
            u8 = u8s.pop(u)
            rinv = rinvs.pop(u)
            an = soft.tile([P, HT, K], BF16, tag="an")
            nc.gpsimd.tensor_mul(
                an[:], u8[:], rinv[:, :, None].to_broadcast((P, HT, K))
            )
            ans[u] = an
            if DBG and u == (0, 0):
                nc.gpsimd.dma_start(out=DBG["an"], in_=an[:])

        def phase_E(u):
            n, h = u
            xbf = xbfs[n]
            an = ans.pop(u)
            if h == 0:
                pes[n] = pe_pool.tile([K, 192], FP32, name="psum_E", tag="pE")
            pe = pes[n]
            for jj in range(HT):
                nc.tensor.matmul(
                    pe[:, 0:D], lhsT=an[:, jj, :], rhs=xbf[:, h * HT + jj, :],
                    start=(h == 0 and jj == 0), stop=(h == 1 and jj == HT - 1),
                    skip_group_check=True,
                )
            # sum_t A[t,k] -> pe[0, 128+k], accumulated over both halves.
            # start=False always: a start=True here would clear the whole
            # PSUM bank's has_written bits and wipe the E accumulation
            # (observed on HW); the first E matmul's start=True clears the
            # bank once per row, covering this region too.
            if SUMA_STEP0:
                sa_out = pe[0:1, 128:160][:, None, :].to_broadcast((1, HT, K))
                nc.tensor.matmul(
                    sa_out, lhsT=ones_col[:], rhs=an[:],
                    start=False, stop=(h == 1), skip_group_check=True,
                )
            else:
                for jj in range(HT):
                    nc.tensor.matmul(
                        pe[0:1, 128:160], lhsT=ones_col[:], rhs=an[:, jj, :],
                        start=False, stop=(h == 1 and jj == HT - 1),
                        skip_group_check=True,
                    )
        sa_sbs = {}   # row -> [1, K] bf16 sum_t A staging

        def finish_a(n):
            # DVE: [1,K] row of sums -> sbuf (emitted at end of DVE queue)
            pe = pes[n]
            if DBG and n == 0:
                scr = outp.tile([K, D], FP32, tag="dbgE")
                nc.vector.tensor_copy(scr[:], pe[:, 0:D])
                nc.gpsimd.dma_start(out=DBG["Eraw"], in_=scr[:])
                scr2 = outp.tile([1, 64], FP32, tag="dbgSA")
                nc.vector.tensor_copy(scr2[:, 0:32], pe[0:1, 128:160])
                nc.gpsimd.dma_start(out=DBG["sumA"], in_=scr2[:, 0:32])
            sa_sb = outp.tile([1, K], BF16, tag="sa")
            nc.vector.tensor_copy(sa_sb[:], pe[0:1, 128:160])
            sa_sbs[n] = sa_sb

        def finish_b(n):
            # PE: [1,K] -> [K,1] column via mini-matmul (next iteration)
            pe = pes[n]
            sa_sb = sa_sbs.pop(n)
            nc.tensor.matmul(
                pe[:, 160:161], lhsT=sa_sb[:], rhs=ones11[:],
                start=True, stop=True, skip_group_check=True,
            )

        def finish_c(n):
            # DVE + DMA: E[k,d] = raw - sumA_k * c[k,d]; store
            pe = pes.pop(n)
            xbfs.pop(n)
            e_sb = outp.tile([K, D], FP32, tag="e")
            nc.vector.scalar_tensor_tensor(
                out=e_sb[:], in0=c_neg[:], scalar=pe[:, 160:161],
                in1=pe[:, 0:D], op0=MULT, op1=ADD,
            )
            nc.sync.dma_start(out[n], e_sb[:])

        # ---------------- software-pipelined main loop ----------------
        # per iteration i (unit u_i):
        #   PE : T(i)            Q+MM2(i-1)     E(i-3)
        #   ACT: EX(i-2)         CP(i)
        #   DVE: RS(i-2)         XSQ(i)   [finish via E(i-3)]
        #   GPS: [loads]         AN(i-2)
        NU = len(units)
        load_row(0, nsplit=2)
        load_row(1)
        fin_next = []   # rows whose finish_b/finish_c run this iteration
        for i, u in enumerate(units):
            n, h = u
            if h == 0 and n + 2 < NP:
                load_row(n + 2)
            # PE queue
            for nf in fin_next:
                finish_b(nf)
            phase_T(u)
            if i >= 1:
                phase_Q(units[i - 1])
            if i >= 3:
                phase_E(units[i - 3])
            # ACT queue
            if i >= 2:
                phase_EX(units[i - 2])
            phase_CP(u)
            # DVE queue
            if i >= 2:
                phase_RS(units[i - 2])
            phase_XSQ(u)
            for nf in fin_next:
                finish_c(nf)
            fin_next = []
            if i >= 3 and units[i - 3][1] == 1:
                finish_a(units[i - 3][0])
                fin_next.append(units[i - 3][0])
            # GPSIMD queue
            if i >= 2:
                phase_AN(units[i - 2])
        # tail (drain lagged phases)
        phase_Q(units[NU - 1])
        phase_EX(units[NU - 2])
        phase_RS(units[NU - 2])
        phase_AN(units[NU - 2])
        phase_E(units[NU - 3])      # (6,1)
        finish_a(NP - 2)
        phase_EX(units[NU - 1])
        phase_RS(units[NU - 1])
        phase_AN(units[NU - 1])
        finish_b(NP - 2)
        phase_E(units[NU - 2])      # (7,0)
        finish_c(NP - 2)
        phase_E(units[NU - 1])      # (7,1)
        finish_a(NP - 1)
        finish_b(NP - 1)
        finish_c(NP - 1)


_NC_CACHE = None


def _get_nc():
    global _NC_CACHE
    if _NC_CACHE is None:
        _NC_CACHE = _build_bass()
    return _NC_CACHE


def kernel(**inputs):
    x = np.ascontiguousarray(np.asarray(inputs["x"], dtype=np.float32))
    cw = np.ascontiguousarray(np.asarray(inputs["codewords"], dtype=np.float32))
    sc = np.ascontiguousarray(np.asarray(inputs["scale"], dtype=np.float32))

    nc = _get_nc()
    cwT = np.ascontiguousarray(cw.T)
    in_maps = [
        {"x": x[i * NP : (i + 1) * NP], "codewords": cw, "codewordsT": cwT,
         "scale": sc}
        for i in range(NCORES)
    ]
    res = bass_utils.run_bass_kernel_spmd(nc, in_maps, core_ids=list(range(NCORES)))
    return np.concatenate([r["out"] for r in res.results], axis=0)


if __name__ == "__main__":
    rng = np.random.default_rng(0)
    ins = {
        "x": rng.standard_normal((N, T, D), dtype=np.float32),
        "codewords": rng.uniform(-0.01, 0.01, (K, D)).astype(np.float32),
        "scale": rng.uniform(-0.01, 0.01, (K,)).astype(np.float32),
    }
    out = kernel(**ins)
    print(out.shape, out.dtype)

    # numpy reference check
    xx = ins["x"]; c = ins["codewords"]; s = ins["scale"]
    sqx = (xx * xx).sum(-1, keepdims=True)
    cross = xx @ c.T
    sqc = (c * c).sum(-1)
    sm = s * (sqx - 2 * cross + sqc)
    sm -= sm.max(-1, keepdims=True)
    A = np.exp(sm); A /= A.sum(-1, keepdims=True)
    E = np.einsum("ntk,ntd->nkd", A, xx) - A.sum(1)[:, :, None] * c
    err = np.abs(out - E).max() / np.abs(E).max()
    print("rel err vs numpy:", err)
